# revision 33
# baseline (speedup 1.0000x reference)
"""Trainium2 Bass kernel for an 8-layer dense MLP (784->512x6->10) + softmax.

Strategy (hardcoded for batch=65536, 8 NeuronCores, pure data parallel):
  - Each core handles 8192 rows of the batch; weights replicated.
  - All matmuls run in fp8 e4m3 with DoubleRow perf mode (K=256 per
    instruction, 2x PE throughput vs fp32r). Activations are stored fp8
    feature-major ([128, ko, BT]); weights are quantized fp8 on host.
    Measured end-to-end fp8 error vs the fp32 reference: 2.6e-3 rel
    (tolerance 2e-2).
  - Dropout masks (jax threefry, key 42) are bit-exactly precomputed on host
    and shipped as {0x00, 0xFF} bytes; applied on-chip as a bitwise AND over
    uint16-viewed fp8 pairs (hits the DVE 2x_1p fast path). The 1/(1-p)
    rescale is folded into the next layer's weights on host.
  - relu(psum + bias) passes are load-balanced across the Scalar (ACT),
    Vector (DVE) and GpSimd (Pool) engines: ACT uses the fused
    activation(Relu, bias=...), DVE/Pool use tensor_scalar(add bias, max 0).
  - Softmax: exp on ACT (bias = b8), class-sum broadcast via a tiny ones
    matmul on the PE (replaces the 3.5us gpsimd all-reduce), fast
    approximate reciprocal, one multiply.
  - Host pre-swizzles every dram tensor so each per-tile DMA is one
    contiguous block per partition.
"""

import numpy as np

BATCH = 65536
D_IN = 784
KO1 = 8                    # 1024 = 8*128 padded input-feature chunks (even for DoubleRow)
D_PAD = KO1 * 128
H = 512
KO = H // 128              # 4 feature chunks for hidden layers
C = 10
C_PAD = 16  # DoubleRow ldweights needs the weight AP chunk step %16 == 0
N_CORES = 8
B_CORE = BATCH // N_CORES  # 8192
BT = 512                   # batch tile (matmul moving free dim)

DROP_LAYERS = (2, 4, 6)
KEEP = {2: 0.8, 4: 0.7, 6: 0.5}

# Engine assignment for the relu(psum+bias) pass of (layer, n-chunk):
# 'a' = Scalar/ACT, 'v' = Vector/DVE, 'p' = GpSimd/Pool.
RELU_ENG = {
    1: "vvvv",
    2: "aaaa",
    3: "vvvv",
    4: "aaaa",
    5: "vvvv",
    6: "aaaa",
    7: "aaaa",
}
# Engine for the dropout mask application of (layer, n-chunk):
# 'v' = DVE bitwise-AND on uint32 views (bitwise ops are DVE+u32 only),
# 'p' = Pool fp8 min: mask byte 0xFF is NaN in e4m3 -> min returns h (keep),
#       0x00 is +0.0 -> min clamps to 0 (drop). Pool cannot touch PSUM, so
#       this is the only way it can help with the dropout work.
AND_ENG = {2: "vvvv", 4: "vvvv", 6: "vvvv"}


def build_bass(b_core: int, debug: bool = False):
    """Build the Bass module for one core processing b_core batch rows."""
    import concourse.mybir as mybir
    import concourse.tile as tile
    from concourse import bacc

    f32 = mybir.dt.float32
    f32r = mybir.dt.float32r
    f8 = mybir.dt.float8e4
    u8 = mybir.dt.uint8
    u32 = mybir.dt.uint32
    AF = mybir.ActivationFunctionType
    ALU = mybir.AluOpType
    DR = mybir.MatmulPerfMode.DoubleRow

    nbt = b_core // BT

    nc = bacc.Bacc("TRN2", target_bir_lowering=False, debug=False)

    # Host-preswizzled layouts: one contiguous block per partition per DMA.
    xT = nc.dram_tensor("xT", [128, nbt, KO1, BT], f8, kind="ExternalInput")
    w_h = {1: nc.dram_tensor("w1", [128, KO1, H], f8, kind="ExternalInput")}
    for l in range(2, 8):
        w_h[l] = nc.dram_tensor(f"w{l}", [128, KO, H], f8, kind="ExternalInput")
    w8_h = nc.dram_tensor("w8", [128, KO, C_PAD], f8, kind="ExternalInput")
    bias17_h = nc.dram_tensor("bias17", [128, 28], f32, kind="ExternalInput")
    b8c_h = nc.dram_tensor("b8c", [128, 1], f32, kind="ExternalInput")
    m_h = {
        l: nc.dram_tensor(f"m{l}", [128, nbt, KO, BT], u8, kind="ExternalInput")
        for l in DROP_LAYERS
    }
    y_h = nc.dram_tensor("yT", [C, b_core], f32, kind="ExternalOutput")
    dbg_h = {}
    if debug:
        for l in range(1, 8):
            dbg_h[l] = nc.dram_tensor(
                f"dbg_h{l}", [128, b_core // BT, KO, BT], f8, kind="ExternalOutput"
            )

    eng_of = {"a": None, "v": None, "p": None}  # filled inside context

    with tile.TileContext(nc) as tc:
        with (
            tc.tile_pool(name="wpool", bufs=1) as wpool,
            tc.tile_pool(name="xpool", bufs=4) as xpool,
            tc.tile_pool(name="hpool", bufs=5) as hpool,
            tc.tile_pool(name="mpool", bufs=4) as mpool,
            tc.tile_pool(name="spool", bufs=3) as spool,
            tc.tile_pool(name="opool", bufs=3) as opool,
            tc.tile_pool(name="psum", bufs=5, space="PSUM") as pp,
            tc.tile_pool(name="psum8", bufs=2, space="PSUM") as pp8,
            tc.tile_pool(name="psums", bufs=1, space="PSUM") as pps,
        ):
            eng_of = {"a": nc.scalar, "v": nc.vector, "p": nc.gpsimd}

            gate = {"inst": None}
            chain = {"prev": None}

            def chained(di):
                if chain["prev"] is not None:
                    tile.add_dep_helper(di.ins, chain["prev"].ins, sync=True)
                chain["prev"] = di
                return di

            def load_x(bt, in_chain=False):
                xt = xpool.tile([128, KO1, BT], f8, tag="xt", name="xt")
                di = nc.sync.dma_start(xt[:], xT.ap()[:, bt, :, :])
                if in_chain:
                    chained(di)
                if gate["inst"] is not None:
                    tile.add_dep_helper(di.ins, gate["inst"], sync=True)
                return xt

            def load_mask(bt, l, in_chain=False):
                m = mpool.tile([128, KO, BT], u8, tag=f"m{l}", name=f"m{l}_t")
                # steady-state masks ride the idle gpsimd queue; startup ones
                # are chained just-in-time into the sync startup stream
                eng = nc.sync if in_chain else nc.gpsimd
                mi = eng.dma_start(m[:], m_h[l].ap()[:, bt, :, :])
                if in_chain:
                    chained(mi)
                if gate["inst"] is not None:
                    tile.add_dep_helper(mi.ins, gate["inst"], sync=True)
                return m

            def load_bt(bt):
                xt = load_x(bt)
                mt = {l: load_mask(bt, l) for l in DROP_LAYERS}
                return xt, mt

            # Warm the PE HAM clock-gate with dummy fp32 matmuls during the
            # initial DMA wait.
            warm_w = wpool.tile([128, 128], f32, tag="warm_w")
            warm_x = wpool.tile([128, BT], f32, tag="warm_x")
            nc.vector.memset(warm_w[:], 0)
            nc.vector.memset(warm_x[:], 0)
            warm_ps = pp.tile([128, BT], f32, tag="ps", name="warm_ps")
            for _ in range(4):
                nc.tensor.matmul(warm_ps[:], lhsT=warm_w[:], rhs=warm_x[:])

            # ones matrix for the softmax class-sum broadcast matmul
            ones10 = wpool.tile([C, C], f32, tag="ones10")
            nc.vector.memset(ones10[:], 1.0)

            # Startup DMAs are chained into a just-in-time serial order on the
            # sync queue: each transfer lands comfortably before its first
            # consumer while never delaying anything needed earlier.
            # Prefetches beyond tiles 0/1 are gated on the last weight DMA.
            w_t = {}

            def load_w(l):
                ko = KO1 if l == 1 else KO
                w_t[l] = wpool.tile([128, ko, H], f8, tag=f"w{l}", name=f"w{l}_t")
                return chained(nc.sync.dma_start(w_t[l][:], w_h[l].ap()[:]))

            xt0 = load_x(0, in_chain=True)
            load_w(1)
            mt0, mt1 = {}, {}
            mt0[2] = load_mask(0, 2, in_chain=True)
            xt1 = load_x(1, in_chain=True)
            load_w(2)
            mt0[4] = load_mask(0, 4, in_chain=True)
            load_w(3)
            mt0[6] = load_mask(0, 6, in_chain=True)
            load_w(4)
            mt1[2] = load_mask(1, 2, in_chain=True)
            load_w(5)
            mt1[4] = load_mask(1, 4, in_chain=True)
            load_w(6)
            mt1[6] = load_mask(1, 6, in_chain=True)
            w7_dma = load_w(7)
            w8_t = wpool.tile([128, KO, C_PAD], f8, tag="w8")
            nc.sync.dma_start(w8_t[:], w8_h.ap()[:])
            bias17_t = wpool.tile([128, 28], f32, tag="bias17")
            nc.sync.dma_start(bias17_t[:], bias17_h.ap())
            b8c_t = wpool.tile([128, 1], f32, tag="b8c")
            nc.sync.dma_start(b8c_t[:], b8c_h.ap())
            gate["inst"] = w7_dma.ins

            def hidden_layer(l, src, mt, bt=None):
                ko_in = KO1 if l == 1 else KO
                hn = hpool.tile([128, KO, BT], f8, tag="h", name="h")
                for n in range(KO):
                    ps = pp.tile([128, BT], f32, tag="ps", name="ps")
                    for kk in range(ko_in // 2):
                        nc.tensor.matmul(
                            ps[:],
                            lhsT=w_t[l][:, 2 * kk : 2 * kk + 2, n * 128 : (n + 1) * 128],
                            rhs=src[:, 2 * kk : 2 * kk + 2, :],
                            start=(kk == 0),
                            stop=(kk == ko_in // 2 - 1),
                            perf_mode=DR,
                        )
                    bias_ap = bias17_t[:, (l - 1) * 4 + n : (l - 1) * 4 + n + 1]
                    e = RELU_ENG[l][n]
                    if e == "a":
                        nc.scalar.activation(hn[:, n, :], ps[:], AF.Relu, bias=bias_ap)
                    else:
                        eng_of[e].tensor_scalar(
                            hn[:, n, :], ps[:], bias_ap, 0.0, ALU.add, ALU.max
                        )
                    if l in DROP_LAYERS:
                        if AND_ENG[l][n] == "v":
                            hv = hn[:, n, :].bitcast(u32)
                            mv = mt[l][:, n, :].bitcast(u32)
                            nc.vector.tensor_tensor(hv, hv, mv, ALU.bitwise_and)
                        else:
                            nc.gpsimd.tensor_tensor(
                                hn[:, n, :],
                                hn[:, n, :],
                                mt[l][:, n, :].bitcast(f8),
                                ALU.min,
                            )
                if debug and bt is not None:
                    nc.sync.dma_start(dbg_h[l].ap()[:, bt, :, :], hn[:])
                return hn

            def final_layer(h, bs, mult_eng="p"):
                # layer 8 (512->10) feature-major [10, BT]; softmax over the
                # partition dim: exp (bias=b8) on ACT, class-sum broadcast via
                # a [10,10] ones matmul on the PE, fast reciprocal + multiply.
                ps8 = pp8.tile([C_PAD, BT], f32, tag="ps8", name="ps8")
                for kk in range(KO // 2):
                    nc.tensor.matmul(
                        ps8[:],
                        lhsT=w8_t[:, 2 * kk : 2 * kk + 2, :],
                        rhs=h[:, 2 * kk : 2 * kk + 2, :],
                        start=(kk == 0),
                        stop=(kk == KO // 2 - 1),
                        perf_mode=DR,
                    )
                ex = spool.tile([C, BT], f32r, tag="ex", name="ex")
                nc.scalar.activation(ex[:], ps8[:C, :], AF.Exp, bias=b8c_t[:C, 0:1])
                pss = pps.tile([C, BT], f32, tag="pss", name="pss")
                nc.tensor.matmul(pss[:], lhsT=ones10[:].bitcast(f32r), rhs=ex[:])
                rs = spool.tile([C, BT], f32, tag="rs", name="rs")
                nc.vector.reciprocal_approx_fast(rs[:], pss[:])
                ot = opool.tile([C, BT], f32, tag="ot", name="ot")
                eng_of[mult_eng].tensor_tensor(ot[:], ex[:].bitcast(f32), rs[:], ALU.mult)
                nc.sync.dma_start(y_h.ap()[:, bs : bs + BT], ot[:])

            # Two-tile software pipeline throughout: while tile A's relu/mask
            # chain drains on ACT/DVE, the PE runs tile B's matmuls, so the PE
            # never stalls at layer boundaries.
            for p0 in range(0, nbt, 2):
                if p0 == 0:
                    hA, mtA, hB, mtB = xt0, mt0, xt1, mt1
                else:
                    hA, mtA = load_bt(p0)
                    hB, mtB = load_bt(p0 + 1)
                for l in range(1, 8):
                    hA = hidden_layer(l, hA, mtA, bt=p0)
                    hB = hidden_layer(l, hB, mtB, bt=p0 + 1)
                last = p0 + 2 >= nbt
                final_layer(hA, p0 * BT, mult_eng="v" if last else "p")
                final_layer(hB, (p0 + 1) * BT, mult_eng="v" if last else "p")

    nc.compile()
    return nc


def host_prepare(inputs: dict, n_cores: int = N_CORES) -> tuple[dict, dict]:
    """Quantize weights/x to fp8, fold dropout scaling, precompute masks,
    pre-swizzle everything into the device layouts.

    Returns (shared_inputs, per_core_varying)."""
    import jax
    import ml_dtypes

    f8 = ml_dtypes.float8_e4m3

    x = np.asarray(inputs["x"], dtype=np.float32)
    batch = x.shape[0]
    b_core = batch // n_cores
    nbt = b_core // BT
    W = {i: np.asarray(inputs[f"W{i}"], dtype=np.float32) for i in range(1, 9)}
    b = {i: np.asarray(inputs[f"b{i}"], dtype=np.float32) for i in range(1, 9)}

    # Dropout masks — bit-exact replication of the reference's PRNG stream,
    # shipped as {0x00, 0xFF} for the on-chip bitwise-AND.
    cpu = jax.devices("cpu")[0]
    with jax.default_device(cpu):
        dk = jax.random.split(jax.random.key(42), 3)
        keeps = {
            l: np.asarray(
                jax.random.bernoulli(dk[i], KEEP[l], (batch, H)), dtype=np.uint8
            )
            * np.uint8(255)
            for i, l in enumerate(DROP_LAYERS)
        }

    # Fold 1/(1-p) into the next layer's weights, then quantize to fp8.
    Wf = dict(W)
    for l in DROP_LAYERS:
        Wf[l + 1] = (W[l + 1] / np.float32(KEEP[l])).astype(np.float32)

    W1p = np.zeros((D_PAD, H), dtype=np.float32)
    W1p[:D_IN] = Wf[1]

    def swz_w(w, ko):  # [ko*128, n] -> [128, ko, n]
        return np.ascontiguousarray(
            w.astype(f8).reshape(ko, 128, -1).transpose(1, 0, 2)
        )

    bias17 = np.empty((128, 28), dtype=np.float32)
    for l in range(1, 8):
        bias17[:, (l - 1) * 4 : l * 4] = b[l].reshape(4, 128).T
    b8c = np.zeros((128, 1), dtype=np.float32)
    b8c[:C, 0] = b[8]

    W8p = np.zeros((H, C_PAD), dtype=np.float32)
    W8p[:, :C] = Wf[8]
    shared = {
        "w1": swz_w(W1p, KO1),
        "w8": swz_w(W8p, KO),
        "bias17": bias17,
        "b8c": b8c,
    }
    for l in range(2, 8):
        shared[f"w{l}"] = swz_w(Wf[l], KO)

    # x: [batch, 784] -> fp8, pad to 1024 features, swizzle to
    # [128, nbt, 8, 512] per core (partition p holds feature ko*128+p).
    x8 = x.astype(f8)
    xTp = np.zeros((D_PAD, batch), dtype=f8)
    xTp[:D_IN] = x8.T

    def swz_act(a, ko, sl):  # [ko*128, batch] -> core slice [128, nbt, ko, BT]
        ac = a[:, sl].reshape(ko, 128, nbt, BT)
        return np.ascontiguousarray(ac.transpose(1, 2, 0, 3))

    per_core = {"xT": [], "m2": [], "m4": [], "m6": []}
    mT = {l: keeps[l].T for l in DROP_LAYERS}
    for c in range(n_cores):
        sl = slice(c * b_core, (c + 1) * b_core)
        per_core["xT"].append(swz_act(xTp, KO1, sl))
        for l in DROP_LAYERS:
            per_core[f"m{l}"].append(swz_act(mT[l], KO, sl))
    return shared, per_core


def run_hw(inputs: dict, trace: bool = False):
    from concourse import bass_utils

    shared, per_core = host_prepare(inputs)
    nc = build_bass(B_CORE)
    in_maps = [
        {**shared, **{k: v[c] for k, v in per_core.items()}} for c in range(N_CORES)
    ]
    res = bass_utils.run_bass_kernel_spmd(
        nc, in_maps, core_ids=list(range(N_CORES)), trace=trace
    )
    out = np.concatenate([np.ascontiguousarray(r["yT"].T) for r in res.results], axis=0)
    return out.astype(np.float32), res


def kernel(**inputs) -> np.ndarray:
    return run_hw(inputs, trace=False)[0]
